# revision 1
# baseline (speedup 1.0000x reference)
"""AdaConv Trainium2 kernel: instance-norm + per-sample depthwise 3x3 (+scale+bias)
+ shared dense 3x3 conv 256->1024, data-parallel over batch on 8 NeuronCores.

Layout (per core = one sample), activations/weights bf16 on the PE path:
  - xt [2][128 ch, 66, 66] bf16 with 1-pixel border, DMA'd in row-halves
    ahead of the conv weights so the stats path starts early.
  - Instance norm: cb0 stats via DVE bn_stats (11 chunks of 6 padded rows,
    combined manually - borders are zero so raw sums = interior sums); cb1
    via two accumulating ACT passes per DMA half. iv = 1/(sqrt(var)+eps) is
    folded into the depthwise tap weights (wsc = w*iv) and the tap-0
    constant bt = bias - mean*iv*sum(w); xt borders are mean-filled so the
    folded normalization maps them to exactly zero.
  - Depthwise 3x3 on the DVE writes one av [128, 66, 66] bf16 tile per cb
    in 8 row-chunks (chunk0 = rows 1..9, then 8-row chunks, no halo
    recompute); av borders pre-zeroed by GPSIMD memset. Emission order makes
    chunk0/cb0 the only gate for the first matmul.
  - Shared 3x3 conv: per pixel-row-block pc, 8 psum banks accumulate 2x9
    tap matmuls (bf16, 512-px moving dim = full PE rate); all cb0 taps for
    the 8 ob blocks are emitted before the cb1 taps so the PE starts as
    soon as cb0's first depthwise chunk lands.
  - ACT evacuates psum (+conv_b) to f32; DMA out o-major [1024, 4096]; the
    final row-block's output DMAs are split in two to shorten the tail.
Host re-lays-out the o-major per-core outputs into [8, 64, 64, 1024].
"""

import os

import numpy as np

import concourse.bacc as bacc
import concourse.mybir as mybir
import concourse.tile as tile
from concourse.bass_utils import run_bass_kernel_spmd

F32 = mybir.dt.float32
BF16 = mybir.dt.bfloat16
AF = mybir.ActivationFunctionType
ALU = mybir.AluOpType

B = 8
H = W = 64
C = 256
CB = 2
OUT = 1024
PW = 66
NPIX = H * W
EPS = 1e-5
N_OB = OUT // 128
N_PC = H // 8
N_CH = 8  # depthwise row chunks


def _chunk_rows(k: int) -> tuple[int, int]:
    """Padded-row range [r0, r1) written by depthwise chunk k."""
    if k == 0:
        return 1, 10
    return 8 * k + 2, min(8 * k + 10, 65)


def build_nc(compile: bool = True):
    nc = bacc.Bacc("TRN2", target_bir_lowering=False, debug=False)

    xt_d = nc.dram_tensor("xt", [CB, 128, PW, PW], BF16, kind="ExternalInput").ap()
    wv_d = nc.dram_tensor("wv", [CB, 128, 9], F32, kind="ExternalInput").ap()
    bias_d = nc.dram_tensor("bias", [CB, 128, 1], F32, kind="ExternalInput").ap()
    cwt_d = nc.dram_tensor("cwt", [CB, 128, 9, OUT], BF16, kind="ExternalInput").ap()
    cbt_d = nc.dram_tensor("cbt", [128, N_OB], F32, kind="ExternalInput").ap()
    out_d = nc.dram_tensor("out", [OUT, NPIX], F32, kind="ExternalOutput").ap()

    with tile.TileContext(nc) as tc:
        with (
            tc.tile_pool(name="res", bufs=1) as RP,
            tc.tile_pool(name="psc", bufs=8, space="PSUM") as PSC,
            tc.tile_pool(name="outp", bufs=4) as OP,
            tc.tile_pool(name="small", bufs=1) as SP,
        ):
            xt = [RP.tile([128, PW, PW], BF16, name=f"xt{i}", tag=f"xt{i}") for i in range(CB)]
            av = [RP.tile([128, PW, PW], BF16, name=f"av{i}", tag=f"av{i}") for i in range(CB)]
            cw = [RP.tile([128, 9, OUT], BF16, name=f"cw{i}", tag=f"cw{i}") for i in range(CB)]
            wv = [SP.tile([128, 9], F32, name=f"wv{i}", tag=f"wv{i}") for i in range(CB)]
            bi = [SP.tile([128, 1], F32, name=f"bi{i}", tag=f"bi{i}") for i in range(CB)]
            cbt = SP.tile([128, N_OB], F32, name="cbt", tag="cbt")
            warm = SP.tile([128, 1], F32, name="warm", tag="warm")
            warm2 = SP.tile([128, 1], F32, name="warm2", tag="warm2")

            # ACT table warm-up (Rsqrt + Identity) while DMAs stream in.
            nc.gpsimd.memset(warm[:, :], 0.0)
            nc.scalar.activation(out=warm2[:, :], in_=warm[:, :], func=AF.Sqrt)
            nc.scalar.activation(out=warm2[:, :], in_=warm[:, :], func=AF.Identity)

            # av borders must read as zero in the shared conv.
            for cb in range(CB):
                nc.gpsimd.memset(av[cb][:, 0 : PW : 65, :], 0.0)
                nc.gpsimd.memset(av[cb][:, :, 0 : PW : 65], 0.0)

            # DMA priority: xt row halves first, then small scalars, then cw.
            for cb in range(CB):
                nc.sync.dma_start(out=xt[cb][:, 0:33, :], in_=xt_d[cb][:, 0:33, :])
                nc.sync.dma_start(out=xt[cb][:, 33:66, :], in_=xt_d[cb][:, 33:66, :])
            for cb in range(CB):
                nc.sync.dma_start(out=wv[cb][:, :], in_=wv_d[cb])
                nc.sync.dma_start(out=bi[cb][:, :], in_=bias_d[cb])
            nc.sync.dma_start(out=cbt[:, :], in_=cbt_d)
            for ob in range(N_OB):
                for cb in range(CB):
                    nc.sync.dma_start(
                        out=cw[cb][:, :, ob * 128 : (ob + 1) * 128],
                        in_=cwt_d[cb][:, :, ob * 128 : (ob + 1) * 128],
                    )

            # ---- instance-norm statistics ----
            # cb0 on DVE via bn_stats; cb1 on ACT (accumulating passes split
            # by DMA half). cb1's second copy pass is emitted after cb0's
            # sqrt chain so the sqrt wins the ACT priority race and cb0's
            # depthwise chunk (which alone gates the first matmul) starts
            # as early as possible.
            scr = RP.tile([128, 33, PW], F32, name="scr", tag="scr")
            s1a = SP.tile([128, 1], F32, name="s1a", tag="s1a")
            s1b = SP.tile([128, 1], F32, name="s1b", tag="s1b")
            q1a = SP.tile([128, 1], F32, name="q1a", tag="q1a")
            q1b = SP.tile([128, 1], F32, name="q1b", tag="q1b")
            nc.scalar.activation(
                out=scr[:, :, :], in_=xt[1][:, 0:33, :], func=AF.Copy,
                accum_out=s1a[:, :],
            )

            bs0 = SP.tile([128, 11, 6], F32, name="bs0", tag="bs0")
            xf0 = xt[0][:, :, :].rearrange("p a b -> p (a b)")
            for q in range(11):
                nc.vector.bn_stats(out=bs0[:, q, :], in_=xf0[:, 396 * q : 396 * (q + 1)])
            wa = [SP.tile([128, 1], F32, name=f"wa{cb}", tag=f"wa{cb}") for cb in range(CB)]
            for cb in range(CB):
                nc.vector.tensor_reduce(
                    out=wa[cb][:, :], in_=wv[cb][:, :], axis=mybir.AxisListType.X,
                    op=ALU.add,
                )

            wvs = [None, None]   # iv-scaled tap weights [128, 9]
            bt0 = [None, None]   # tap-0 constant: bias + nm * sum(w) [128, 1]

            def fold_params(cb, mean, var):
                """sqrt -> iv -> folded depthwise params + xt border mean-fill."""
                iv = SP.tile([128, 1], F32, name=f"iv{cb}", tag=f"iv{cb}")
                nm = SP.tile([128, 1], F32, name=f"nm{cb}", tag=f"nm{cb}")
                wsc = SP.tile([128, 9], F32, name=f"wsc{cb}", tag=f"wsc{cb}")
                bt = SP.tile([128, 1], F32, name=f"bt{cb}", tag=f"bt{cb}")
                std = SP.tile([128, 1], F32, name=f"std{cb}", tag=f"std{cb}")
                sd = SP.tile([128, 1], F32, name=f"sd{cb}", tag=f"sd{cb}")
                nc.scalar.activation(out=std[:, :], in_=var[:, :], func=AF.Sqrt)
                nc.vector.tensor_scalar_add(sd[:, :], std[:, :], EPS)
                nc.vector.reciprocal(iv[:, :], sd[:, :])
                nc.vector.tensor_scalar(
                    out=nm[:, :], in0=mean[:, :], scalar1=iv[:, :], scalar2=-1.0,
                    op0=ALU.mult, op1=ALU.mult,
                )
                nc.vector.tensor_scalar_mul(wsc[:, :], wv[cb][:, :], iv[:, :])
                nc.vector.scalar_tensor_tensor(
                    out=bt[:, :], in0=nm[:, :], scalar=wa[cb][:, :], in1=bi[cb][:, :],
                    op0=ALU.mult, op1=ALU.add,
                )
                border_rows = xt[cb][:, 0 : PW : 65, :]
                nc.scalar.activation(
                    out=border_rows, in_=border_rows,
                    func=AF.Identity, bias=mean[:, :], scale=0.0,
                )
                border_cols = xt[cb][:, 1:65, 0 : PW : 65]
                nc.scalar.activation(
                    out=border_cols, in_=border_cols,
                    func=AF.Identity, bias=mean[:, :], scale=0.0,
                )
                wvs[cb] = wsc
                bt0[cb] = bt

            def dw_chunk(k, cb, col_splits=1):
                # col_splits=2 makes two independent tap chains, so the
                # scheduler fills dependency bubbles with this chunk's own
                # work instead of a later chunk's (which would delay the
                # matmul gated on this chunk).
                r0, r1 = _chunk_rows(k)
                cols = [(1, 65)] if col_splits == 1 else [(1, 33), (33, 65)]
                for t in range(9):
                    ty, tx = t // 3, t % 3
                    for c0, c1 in cols:
                        dst = av[cb][:, r0:r1, c0:c1]
                        src = xt[cb][:, r0 + ty - 1 : r1 + ty - 1, tx + c0 - 1 : tx + c1 - 1]
                        if t == 0:
                            nc.vector.tensor_scalar(
                                out=dst, in0=src,
                                scalar1=wvs[cb][:, 0:1], scalar2=bt0[cb][:, :],
                                op0=ALU.mult, op1=ALU.add,
                            )
                        else:
                            nc.vector.scalar_tensor_tensor(
                                out=dst, in0=src, scalar=wvs[cb][:, t : t + 1],
                                in1=dst, op0=ALU.mult, op1=ALU.add,
                            )

            # cb0 math: combine the 22 equal-count (198) half-chunk stats:
            # sum = 198*sum(means); sumsq = sum(c*var) + 198*sum(means^2);
            # interior mean/E[x^2] = sums / 4096 (borders are zero).
            CNT = 198.0
            sm = SP.tile([128, 1], F32, name="sm0", tag="sm0")
            scv = SP.tile([128, 1], F32, name="scv0", tag="scv0")
            ms2 = SP.tile([128, 11, 2], F32, name="ms20", tag="ms20")
            sm2 = SP.tile([128, 1], F32, name="sm20", tag="sm20")
            mean0 = SP.tile([128, 1], F32, name="mean0", tag="mean0")
            e2a = SP.tile([128, 1], F32, name="e2a0", tag="e2a0")
            e20 = SP.tile([128, 1], F32, name="e20", tag="e20")
            msq0 = SP.tile([128, 1], F32, name="msq0", tag="msq0")
            var0 = SP.tile([128, 1], F32, name="var0", tag="var0")
            means = bs0[:, :, 1:6:3]
            cvars = bs0[:, :, 2:6:3]
            nc.vector.tensor_reduce(
                out=sm[:, :], in_=means, axis=mybir.AxisListType.XY, op=ALU.add,
            )
            nc.vector.tensor_reduce(
                out=scv[:, :], in_=cvars, axis=mybir.AxisListType.XY, op=ALU.add,
            )
            nc.vector.tensor_mul(ms2[:, :, :], means, means)
            nc.vector.tensor_reduce(
                out=sm2[:, :], in_=ms2[:, :, :], axis=mybir.AxisListType.XY, op=ALU.add,
            )
            nc.vector.tensor_scalar_mul(mean0[:, :], sm[:, :], CNT / NPIX)
            nc.vector.tensor_scalar_mul(e2a[:, :], scv[:, :], 1.0 / NPIX)
            nc.vector.scalar_tensor_tensor(
                out=e20[:, :], in0=sm2[:, :], scalar=CNT / NPIX, in1=e2a[:, :],
                op0=ALU.mult, op1=ALU.add,
            )
            nc.vector.tensor_mul(msq0[:, :], mean0[:, :], mean0[:, :])
            nc.vector.tensor_sub(var0[:, :], e20[:, :], msq0[:, :])
            with tc.high_priority():
                fold_params(0, mean0, var0)

            # remaining cb1 stat passes (ACT), after cb0's sqrt in priority
            nc.scalar.activation(
                out=scr[:, :, :], in_=xt[1][:, 33:66, :], func=AF.Copy,
                accum_out=s1b[:, :],
            )
            nc.scalar.activation(
                out=scr[:, :, :], in_=xt[1][:, 0:33, :], func=AF.Square,
                accum_out=q1a[:, :],
            )
            nc.scalar.activation(
                out=scr[:, :, :], in_=xt[1][:, 33:66, :], func=AF.Square,
                accum_out=q1b[:, :],
            )

            dw_chunk(0, 0, col_splits=2)

            # cb1 math from the ACT accumulators
            sum1 = SP.tile([128, 1], F32, name="sum1", tag="sum1")
            mean1 = SP.tile([128, 1], F32, name="mean1", tag="mean1")
            ssq1 = SP.tile([128, 1], F32, name="ssq1", tag="ssq1")
            e21 = SP.tile([128, 1], F32, name="e21", tag="e21")
            msq1 = SP.tile([128, 1], F32, name="msq1", tag="msq1")
            var1 = SP.tile([128, 1], F32, name="var1", tag="var1")
            nc.vector.tensor_add(sum1[:, :], s1a[:, :], s1b[:, :])
            nc.vector.tensor_scalar_mul(mean1[:, :], sum1[:, :], 1.0 / NPIX)
            nc.vector.tensor_add(ssq1[:, :], q1a[:, :], q1b[:, :])
            nc.vector.tensor_scalar_mul(e21[:, :], ssq1[:, :], 1.0 / NPIX)
            nc.vector.tensor_mul(msq1[:, :], mean1[:, :], mean1[:, :])
            nc.vector.tensor_sub(var1[:, :], e21[:, :], msq1[:, :])
            fold_params(1, mean1, var1)

            dw_chunk(0, 1, col_splits=2)
            for k in range(1, N_CH):
                dw_chunk(k, 0)
                dw_chunk(k, 1)

            # ---- shared 3x3 conv 256 -> 1024 (+conv_b) on the PE ----
            for pc in range(N_PC):
                ps = [
                    PSC.tile([128, 8, 64], F32, name=f"psc{ob}_{pc}", tag="psc")
                    for ob in range(N_OB)
                ]
                for cb in range(CB):
                    for ob in range(N_OB):
                        for t in range(9):
                            ty, tx = t // 3, t % 3
                            nc.tensor.matmul(
                                out=ps[ob][:, :, :],
                                lhsT=cw[cb][:, t, ob * 128 : (ob + 1) * 128],
                                rhs=av[cb][:, 8 * pc + ty : 8 * pc + ty + 8, tx : tx + 64],
                                start=(cb == 0 and t == 0),
                                stop=(cb == 1 and t == 8),
                            )
                        if cb == 1:
                            ot = OP.tile([128, 8, 64], F32, name=f"ot{ob}_{pc}", tag="ot")
                            nc.scalar.activation(
                                out=ot[:, :, :], in_=ps[ob][:, :, :],
                                func=AF.Identity, bias=cbt[:, ob : ob + 1], scale=1.0,
                            )
                            orow = out_d[ob * 128 : (ob + 1) * 128]
                            if pc == N_PC - 1:
                                nc.sync.dma_start(
                                    out=orow[:, 512 * pc : 512 * pc + 256],
                                    in_=ot[:, 0:4, :],
                                )
                                nc.sync.dma_start(
                                    out=orow[:, 512 * pc + 256 : 512 * (pc + 1)],
                                    in_=ot[:, 4:8, :],
                                )
                            else:
                                nc.sync.dma_start(
                                    out=orow[:, 512 * pc : 512 * (pc + 1)],
                                    in_=ot[:, :, :],
                                )

    if compile:
        nc.compile()
    return nc


def make_in_maps(inputs: dict) -> list[dict]:
    import ml_dtypes

    x = np.asarray(inputs["x"], dtype=np.float32)
    ws = np.asarray(inputs["w_spatial"], dtype=np.float32)
    wp = np.asarray(inputs["w_pointwise"], dtype=np.float32)
    bias = np.asarray(inputs["bias"], dtype=np.float32)
    conv_w = np.asarray(inputs["conv_w"], dtype=np.float32)
    conv_b = np.asarray(inputs["conv_b"], dtype=np.float32)

    xt = np.zeros((B, CB, 128, PW, PW), np.float32)
    xt[:, :, :, 1:65, 1:65] = x.transpose(0, 3, 1, 2).reshape(B, CB, 128, H, W)
    xt = xt.astype(ml_dtypes.bfloat16)

    weff = (ws[:, :, :, 0, :] * wp[:, 0, 0, 0, :][:, None, None, :]).reshape(B, 9, C)
    wv = np.ascontiguousarray(weff.reshape(B, 9, CB, 128).transpose(0, 2, 3, 1))

    bias_r = np.ascontiguousarray(bias.reshape(B, CB, 128, 1))
    cwt = conv_w.reshape(9, CB, 128, OUT).transpose(1, 2, 0, 3)
    cwt = np.ascontiguousarray(cwt.astype(ml_dtypes.bfloat16))
    cbt = np.ascontiguousarray(conv_b.reshape(N_OB, 128).T)

    return [
        {
            "xt": np.ascontiguousarray(xt[b]),
            "wv": wv[b],
            "bias": bias_r[b],
            "cwt": cwt,
            "cbt": cbt,
        }
        for b in range(B)
    ]


def gather(results: list[dict]) -> np.ndarray:
    outs = []
    for b in range(B):
        o = np.asarray(results[b]["out"])
        outs.append(o.reshape(OUT, H, W).transpose(1, 2, 0))
    return np.ascontiguousarray(np.stack(outs).astype(np.float32))


_STATE = {}


def _get_nc():
    if "nc" not in _STATE:
        _STATE["nc"] = build_nc()
    return _STATE["nc"]


def kernel(**inputs) -> np.ndarray:
    nc = _get_nc()
    in_maps = make_in_maps(inputs)
    last_err = None
    for _attempt in range(3):
        try:
            res = run_bass_kernel_spmd(nc, in_maps, core_ids=list(range(B)))
            return gather(res.results)
        except Exception as e:  # transient device-unrecoverable seen on 1st exec
            last_err = e
    raise last_err

